# revision 5
# baseline (speedup 1.0000x reference)
"""Trainium2 Bass kernel for nn_AttentionMLP (B=4, S=4096, two attention+MLP stages).

Sharding: 8 cores = 4 batches x 2 sequence-halves. Each core computes its
2048 query rows end-to-end; pairwise AllGathers (chunked, pipelined)
exchange the stage-1 output halves so stage 2 attends over the full
sequence.

v4 (vs v2 baseline):
  - Softmax weights quantized to fp8 on the ACT-routed score groups
    (e5m2 stage 1 / e4m3 stage 2, with a fixed exp offset c per stage so
    the weight range fits fp8; the offset cancels in the softmax ratio).
    attn@V for those blocks runs as fp8 DoubleRow matmuls (2 key blocks
    per PE pass) -> ~2x AV throughput. V is fp8-e4m3 for DoubleRow
    blocks, bf16 for the DVE-routed (Schraudolph) blocks; numerator and
    denominator share the quantized weights (numpy-validated ~6e-3).
  - Score groups are 2 key blocks; exp routing alternates ACT/DVE per
    group so both engines stream the softmax concurrently and the sA
    PSUM ring (2 bufs) never waits on a backed-up engine queue.
  - DVE runs (almost) only exp + PSUM drains; ELU-relu and the 1/Z
    drain moved to ACT, the ELU-combine and 1/Z broadcast-multiply to
    the Pool engine (SBUF-only operands).
  - The last chunk's MLP is deferred into the NEXT stage's chunk-0 loop
    (pre_mlp hook) and stage-2 chunk 0 processes key blocks in a
    permuted slice order so the late AllGather slices (3, 7) are needed
    last: no cross-stage pipeline bubble.
"""

import numpy as np
from contextlib import ExitStack

import concourse.bass as bass
import concourse.tile as tile
from concourse import bacc, mybir
from concourse import bass_utils

F32 = mybir.dt.float32
F32R = mybir.dt.float32r
BF16 = mybir.dt.bfloat16
FP8E4 = mybir.dt.float8e4
FP8E5 = mybir.dt.float8e5
I32 = mybir.dt.int32
I16 = mybir.dt.int16
EXP = mybir.ActivationFunctionType.Exp
RELU = mybir.ActivationFunctionType.Relu
ADD = mybir.AluOpType.add
MIN = mybir.AluOpType.min
MAX = mybir.AluOpType.max
MULT = mybir.AluOpType.mult
DR = mybir.MatmulPerfMode.DoubleRow

# Schraudolph fast-exp: exp(x) ~ bitcast_f32(i32(A*x + B)); ~3% sawtooth
# rel err per weight, which softmax averaging damps on the output.
SCH_A = 12102203.1616 / 65536.0
SCH_B = 1064986823.0 / 65536.0
# Score-exp groups routed to DVE (Schraudolph/bf16): odd groups; even
# groups go to ACT (exp -> fp8 -> DoubleRow AV). The odd/even split is
# invariant under the slice-level block permutation below.
DVE_GIS = tuple(range(1, 16, 2))
# exp offset per stage: weights are exp(score - c). Stage-1 scores reach
# ~14.4 -> c=4 keeps exp <= 3.3e4 (e5m2 max 57344). Stage-2 scores are
# tiny (max ~0.75) -> c=0 and e4m3 weights.
C_OFF = (4.0, 0.0)
WDT = (FP8E5, FP8E4)

N_CORES = 8
B, S, D = 4, 4096, 64
R = S // 2            # own query rows per core
HD = 256
NCK = R // 512        # si-chunks per core (4 x 512)
NJB = S // 128        # key blocks (32 x 128)
NG = NJB // 2         # score groups per chunk (16 x 2 blocks)
VP = 80               # padded fp8 vA row pitch (DoubleRow needs step%16==0)

# f32r weight pack (col offsets in f32 words)
# region A (partitions 0-63, one 576-col block per stage): wq_dup|wk_dup|wv|w1t
WQ, WK, WV, W1T = 0, 128, 256, 320
RA = 576
W2T0 = 2 * RA                    # region B: [128, 128] per stage
WB = W2T0 + 256
# f32 bias pack
B1C0 = 0                         # [128, 2] per stage -> cols 0..3
B2C1 = 4                         # [64, 1] stage-1 b2_eff (per-partition)
B2R2 = 5                         # [128, 64] stage-2 b2_eff (replicated)
NEGC = B2R2 + 64                 # [128, 2] -C_OFF per stage
BF32C = NEGC + 2


def build_nc(n_cores=N_CORES, reps=1, exch_chunks=NCK):
    nc = bacc.Bacc("TRN2", target_bir_lowering=False, debug=False,
                   num_devices=n_cores)

    xT_d = nc.dram_tensor("xT", [64, S], F32R, kind="ExternalInput").ap()
    w_d = nc.dram_tensor("wpack", [128, WB], F32R, kind="ExternalInput").ap()
    b_d = nc.dram_tensor("bias32", [128, BF32C], F32,
                         kind="ExternalInput").ap()
    out_d = nc.dram_tensor("out1", [R, 64], F32, kind="ExternalOutput").ap()

    with tile.TileContext(nc) as tc, ExitStack() as ctx:
        consts = ctx.enter_context(tc.tile_pool(name="consts", bufs=1))
        sb = ctx.enter_context(tc.tile_pool(name="sb", bufs=1))
        ps = ctx.enter_context(tc.tile_pool(name="ps", bufs=2, space="PSUM"))
        dram = ctx.enter_context(tc.tile_pool(name="dram", bufs=1,
                                              space="DRAM"))

        wt = consts.tile([128, WB], F32R)
        nc.sync.dma_start(wt[:, 0:RA], w_d[:, 0:RA])
        nc.gpsimd.dma_start(wt[:, RA:WB], w_d[:, RA:WB])
        bt = consts.tile([128, BF32C], F32)
        nc.gpsimd.dma_start(bt[:], b_d[:])
        dma_engines = [nc.sync, nc.gpsimd, nc.sync]

        for _rep in range(reps):
            _body(nc, sb, ps, dram, wt, bt, dma_engines,
                  xT_d, out_d, _rep, n_cores)

    nc.compile()
    return nc


def _body(nc, sb, ps, dram, wt, bt, dma_engines, xT_d, out_d, rep, n_cores):
    xT = sb.tile([64, S], F32R, tag="xt", bufs=2, name=f"xT_{rep}")
    for n in range(8):
        dma_engines[n % 3].dma_start(xT[:, n * 512:(n + 1) * 512],
                                     xT_d[:, n * 512:(n + 1) * 512])
    outT = sb.tile([64, R], F32R, tag="outT", bufs=2, name=f"outT_{rep}")
    xT2 = sb.tile([64, S], F32R, tag="xt", bufs=2, name=f"xT2_{rep}")

    def alloc_proj(sfx):
        # qT/kT duplicated across partition halves (row-packed score tiles)
        qT = sb.tile([128, R], F32R, tag=f"qT{sfx}", name=f"qT{sfx}_{rep}")
        kT = sb.tile([128, S], F32R, tag=f"kT{sfx}", name=f"kT{sfx}_{rep}")
        vA8 = sb.tile([128, NJB, VP], FP8E4, tag=f"vA8{sfx}",
                      name=f"vA8{sfx}_{rep}")
        vA16 = sb.tile([128, NJB, 65], BF16, tag=f"vA16{sfx}",
                       name=f"vA16{sfx}_{rep}")
        nc.vector.memset(vA8[:, :, 64:65], 1.0)
        nc.vector.memset(vA16[:, :, 64:65], 1.0)
        return qT, kT, vA8, vA16

    qT1, kT1, vA81, vA161 = alloc_proj(0)
    qT2, kT2, vA82, vA162 = alloc_proj(1)

    def is_dve(b):
        # block b's exp route; invariant across chunks (incl. the permuted
        # stage-2 chunk 0): group parity = (b mod 4) // 2.
        return (b % 4) >= 2

    # --- projection emitters ------------------------------------------------
    def emit_k(sfx, kT, src, sl):
        wsl = wt[0:64, sfx * RA:(sfx + 1) * RA]
        pk = ps.tile([128, 512], F32, tag="mlp", bufs=2)
        nc.tensor.matmul(pk[:], wsl[:, WK:WK + 128],
                         src[:, sl * 512:(sl + 1) * 512],
                         start=True, stop=True)
        nc.scalar.copy(kT[:, sl * 512:(sl + 1) * 512], pk[:])

    def emit_q(sfx, qT, src, sl):
        wsl = wt[0:64, sfx * RA:(sfx + 1) * RA]
        pq = ps.tile([128, 512], F32, tag="mlp", bufs=2)
        nc.tensor.matmul(pq[:], wsl[:, WQ:WQ + 128],
                         src[:, sl * 512:(sl + 1) * 512],
                         start=True, stop=True)
        nc.vector.tensor_copy(qT[:, sl * 512:(sl + 1) * 512], pq[:])

    def emit_v(sfx, vA8, vA16, src, sl, jb0):
        wsl = wt[0:64, sfx * RA:(sfx + 1) * RA]
        pv = ps.tile([128, 4, 64], F32, tag="mlp", bufs=2)
        for b in range(4):
            nc.tensor.matmul(pv[:, b, :],
                             src[:, sl * 512 + b * 128:sl * 512 + (b + 1) * 128],
                             wsl[:, WV:WV + 64], start=True, stop=True)
        # halves of the quad route to fp8 (ACT/DoubleRow) or bf16 (DVE)
        nc.vector.tensor_copy(vA8[:, jb0:jb0 + 2, 0:64], pv[:, 0:2, :])
        nc.vector.tensor_copy(vA16[:, jb0 + 2:jb0 + 4, 0:64], pv[:, 2:4, :])

    def proj_unit(sfx, qT, kT, vA8, vA16, src, sl, with_q=True):
        def fn():
            emit_k(sfx, kT, src, sl)
            emit_v(sfx, vA8, vA16, src, sl, 4 * sl)
            if with_q and sl < NCK:
                emit_q(sfx, qT, src, sl)
        return fn

    # --- stage-1 -> stage-2 exchange ---------------------------------------
    bounce_ins = [dram.tile([64, 512], F32R, name=f"bi_{rep}_{n}",
                            tag=f"bi{n}") for n in range(NCK)]
    bounce_outs = [dram.tile([2, 64, 512], F32R, name=f"bo_{rep}_{n}",
                             tag=f"bo{n}") for n in range(NCK)]

    def exchange(n):
        nc.sync.dma_start(bounce_ins[n][:], outT[:, n * 512:(n + 1) * 512])
        if n_cores > 1:
            nc.gpsimd.collective_compute(
                "AllGather", mybir.AluOpType.bypass,
                replica_groups=[[0, 1], [2, 3], [4, 5], [6, 7]],
                ins=[bounce_ins[n][:].opt()],
                outs=[bounce_outs[n][:].opt()])
        else:
            for m in range(2):
                nc.sync.dma_start(bounce_outs[n][m], bounce_ins[n][:])
        for m in range(2):
            dma_engines[(m * NCK + n) % 3].dma_start(
                xT2[:, m * R + n * 512:m * R + (n + 1) * 512],
                bounce_outs[n][m])

    # --- one attention+MLP stage -------------------------------------------
    def stage(sfx, qT, kT, vA8, vA16, group_emits, write_out, after_mlp,
              pre_mlp=None, order0=None, defer_last=False):
        wsl = wt[0:64, sfx * RA:(sfx + 1) * RA]
        w2t = wt[:, W2T0 + sfx * 128:W2T0 + (sfx + 1) * 128]
        negc = bt[:, NEGC + sfx:NEGC + sfx + 1]
        schb = SCH_B - SCH_A * C_OFF[sfx]
        wdt = WDT[sfx]
        aTs = [None] * NCK

        def mlp(n):
            # elu(x)+1 = max(x,0) + min(exp(x),1); bias-adds fused on ACT
            aT = aTs[n]
            r = sb.tile([128, 1024], F32, tag="r", bufs=2)
            e = sb.tile([128, 1024], F32, tag="e", bufs=2)
            em = sb.tile([128, 1024], F32, tag="em", bufs=2)
            hT = sb.tile([128, 1024], F32R, tag="hT", bufs=2)
            for j in range(2):
                ph = ps.tile([128, 512], F32, tag="mlp", bufs=2)
                nc.tensor.matmul(ph[:],
                                 wsl[:, W1T + j * 128:W1T + (j + 1) * 128],
                                 aT[:], start=True, stop=True)
                b1j = bt[:, sfx * 2 + j:sfx * 2 + j + 1]
                jsl = slice(j * 512, (j + 1) * 512)
                nc.scalar.activation(r[:, jsl], ph[:], RELU, bias=b1j)
                nc.scalar.activation(e[:, jsl], ph[:], EXP, bias=b1j)
                nc.gpsimd.tensor_scalar_min(em[:, jsl], e[:, jsl], 1.0)
                nc.gpsimd.tensor_add(hT[:, jsl], em[:, jsl], r[:, jsl])
            write_out(n, hT, w2t)
            if after_mlp is not None:
                after_mlp(n)

        for n in range(NCK):
            order = order0 if (n == 0 and order0 is not None) \
                else list(range(NJB))
            av_box = [None]

            def emit_av(ex, jb, gi):
                if av_box[0] is None:
                    av_box[0] = ps.tile([65, 512], F32, tag="av", bufs=2,
                                        name=f"av_{rep}_{sfx}_{n}")
                start = gi == 0
                stop = gi == NG - 1
                if gi in DVE_GIS:
                    for i in range(2):
                        nc.tensor.matmul(av_box[0][:],
                                         vA16[:, jb + i, :],
                                         ex[:, i, :].bitcast(BF16),
                                         start=start and i == 0,
                                         stop=stop and i == 1)
                else:
                    nc.tensor.matmul(av_box[0][:], vA8[:, jb:jb + 2, 0:65],
                                     ex[:, 0:2, :], start=start, stop=stop,
                                     perf_mode=DR)

            pend = None
            for gi in range(NG):
                jb = order[2 * gi]
                assert order[2 * gi + 1] == jb + 1
                for fn in group_emits.get((n, gi), ()):
                    fn()
                st = ps.tile([128, 2, 512], F32, tag="sA", bufs=2)
                for i in range(2):
                    h = (jb + i) % 2
                    nc.tensor.matmul(
                        st[:, i, :],
                        kT[h * 64:(h + 1) * 64,
                           (jb + i) * 128:(jb + i + 1) * 128],
                        qT[h * 64:(h + 1) * 64, n * 512:(n + 1) * 512],
                        start=True, stop=True, tile_position=(h * 64, 0))
                if gi in DVE_GIS:
                    exi = sb.tile([128, 2, 512], I16, tag="expi", bufs=2)
                    nc.vector.tensor_scalar(exi[:], st[:], SCH_A, schb,
                                            op0=MULT, op1=ADD)
                    pend_t = (exi, jb, gi)
                else:
                    ex = sb.tile([128, 2, 512], wdt, tag="exp", bufs=3)
                    nc.scalar.activation(ex[:], st[:], EXP, bias=negc)
                    pend_t = (ex, jb, gi)
                if gi == 3:
                    if n > 0:
                        mlp(n - 1)
                    elif pre_mlp is not None:
                        pre_mlp()
                if pend is not None:
                    emit_av(*pend)
                pend = pend_t
            emit_av(*pend)
            av = av_box[0]

            # normalize: aT = av[0:64] / av[64]
            rs = sb.tile([1, 512], F32, tag="rs", bufs=2)
            nc.vector.tensor_copy(rs[:], av[64:65, :])
            rr = sb.tile([1, 512], F32, tag="rr", bufs=2)
            nc.vector.reciprocal_approx_fast(rr[:], rs[:])
            rb = sb.tile([64, 512], F32, tag="rb", bufs=2)
            nc.gpsimd.partition_broadcast(rb[:], rr[:])
            araw = sb.tile([64, 512], F32, tag="araw", bufs=2)
            nc.scalar.copy(araw[:], av[0:64, :])
            aT = sb.tile([64, 512], F32R, tag="aT", bufs=2)
            nc.gpsimd.tensor_mul(aT[:], araw[:], rb[:])
            aTs[n] = aT
        if defer_last:
            return lambda: mlp(NCK - 1)
        mlp(NCK - 1)
        return None

    # --- stage 1 ------------------------------------------------------------
    def write_out1(n, hT, w2t):
        po = ps.tile([64, 512], F32, tag="mlp", bufs=2)
        nc.tensor.matmul(po[:], w2t[:, 0:64], hT[:, 0:512],
                         start=True, stop=False)
        nc.tensor.matmul(po[:], w2t[:, 64:128], hT[:, 512:1024],
                         start=False, stop=True)
        nc.vector.tensor_scalar_add(outT[:, n * 512:(n + 1) * 512], po[:],
                                    bt[0:64, B2C1:B2C1 + 1])

    def after_mlp1(n):
        exchange(n)
        emit_q(1, qT2, outT, n)

    # bootstrap: first two projection slices before the chunk loop, the rest
    # interleaved into chunk 0's groups.
    proj_unit(0, qT1, kT1, vA81, vA161, xT, 0)()
    proj_unit(0, qT1, kT1, vA81, vA161, xT, 1)()
    emits1 = {(0, g): [proj_unit(0, qT1, kT1, vA81, vA161, xT, g + 2)]
              for g in range(6)}
    mlp1_last = stage(0, qT1, kT1, vA81, vA161, emits1, write_out1,
                      after_mlp1, defer_last=True)

    # --- stage 2 ------------------------------------------------------------
    def write_out2(n, hT, w2t):
        for ss in range(4):
            po2 = ps.tile([128, 64], F32, tag="mlp", bufs=2)
            nc.tensor.matmul(po2[:], hT[:, ss * 128:(ss + 1) * 128],
                             w2t[:, 0:64], start=True, stop=False)
            nc.tensor.matmul(po2[:], hT[:, 512 + ss * 128:512 + (ss + 1) * 128],
                             w2t[:, 64:128], start=False, stop=True)
            fin = sb.tile([128, 64], F32, tag="fin", bufs=2)
            nc.vector.tensor_add(fin[:], po2[:], bt[:, B2R2:B2R2 + 64])
            row0 = n * 512 + ss * 128
            nc.sync.dma_start(out_d[row0:row0 + 128, :], fin[:])

    # stage-1's last-chunk MLP runs inside stage-2's chunk-0 pipeline (at
    # gi==3), which also emits exchange(3). Chunk 0 therefore processes the
    # key blocks slice-permuted so slices 3 and 7 (delivered by that late
    # exchange) are needed last.
    SL_ORDER = [0, 1, 2, 4, 5, 6, 3, 7]
    order0 = [4 * s + i for s in SL_ORDER for i in range(4)]
    proj_unit(1, qT2, kT2, vA82, vA162, xT2, 0, with_q=False)()
    proj_unit(1, qT2, kT2, vA82, vA162, xT2, 1, with_q=False)()
    pslot = {0: 2, 1: 4, 2: 5, 4: 6, 5: 3, 6: 7}
    emits2 = {(0, g): [proj_unit(1, qT2, kT2, vA82, vA162, xT2, sl,
                                 with_q=False)]
              for g, sl in pslot.items()}
    stage(1, qT2, kT2, vA82, vA162, emits2, write_out2, None,
          pre_mlp=mlp1_last, order0=order0)


def prep_inputs(x, q, k, v, q1, k1, v1, W1, b1, W2, b2, W11, b11, W22, b22):
    """Returns per-core in_maps for run_bass_kernel_spmd."""
    f = np.float32

    def cast(a):
        return np.ascontiguousarray(np.asarray(a), dtype=f)

    scale = f(0.125)                      # 1/sqrt(QD), folded into wq
    wpack = np.zeros((128, WB), dtype=f)
    bias32 = np.zeros((128, BF32C), dtype=f)
    for sfx, (qq, kk, vv, W1_, b1_, W2_, b2_) in enumerate(
            [(q, k, v, W1, b1, W2, b2), (q1, k1, v1, W11, b11, W22, b22)]):
        c0 = RA * sfx
        wq = cast(qq) * scale
        wpack[0:64, c0 + WQ:c0 + WQ + 64] = wq
        wpack[0:64, c0 + WQ + 64:c0 + WQ + 128] = wq
        wpack[0:64, c0 + WK:c0 + WK + 64] = cast(kk)
        wpack[0:64, c0 + WK + 64:c0 + WK + 128] = cast(kk)
        wpack[0:64, c0 + WV:c0 + WV + 64] = cast(vv)
        wpack[0:64, c0 + W1T:c0 + W1T + HD] = cast(W1_).T
        w2T = cast(W2_).T                                 # [HD, 64]
        for j in range(2):
            wpack[:, W2T0 + sfx * 128 + j * 64:
                  W2T0 + sfx * 128 + (j + 1) * 64] = w2T[j * 128:(j + 1) * 128]
            bias32[:, sfx * 2 + j] = cast(b1_)[j * 128:(j + 1) * 128]
        b2e = cast(b2_) - cast(W2_).sum(axis=1)           # ELU +1 fold
        if sfx == 0:
            bias32[0:64, B2C1] = b2e
        else:
            bias32[:, B2R2:B2R2 + 64] = np.tile(b2e[None, :], (128, 1))
        bias32[:, NEGC + sfx] = -C_OFF[sfx]

    in_maps = []
    xc = cast(x)
    for c in range(N_CORES):
        b, h = c // 2, c % 2
        xb = xc[b]                      # [S, 64]
        if h == 1:                      # own half first
            xb = np.concatenate([xb[R:], xb[:R]], axis=0)
        in_maps.append({"xT": np.ascontiguousarray(xb.T),
                        "wpack": wpack, "bias32": bias32})
    return in_maps


_NC_CACHE = None


def kernel(**inputs) -> np.ndarray:
    global _NC_CACHE
    if _NC_CACHE is None:
        _NC_CACHE = build_nc()
    nc = _NC_CACHE
    in_maps = prep_inputs(**inputs)
    res = bass_utils.run_bass_kernel_spmd(nc, in_maps,
                                          core_ids=list(range(N_CORES)))
    out = np.empty((B, S, 64), dtype=np.float32)
    for c in range(N_CORES):
        b, h = c // 2, c % 2
        out[b, h * R:(h + 1) * R, :] = res.results[c]["out1"]
    return out


# revision 6
# speedup vs baseline: 2.6633x; 2.6633x over previous
"""Trainium2 Bass kernel for nn_AttentionMLP (B=4, S=4096, two attention+MLP stages).

Sharding: 8 cores = 4 batches x 2 sequence-halves. Each core computes its
2048 query rows end-to-end; pairwise AllGathers (chunked, pipelined)
exchange the stage-1 output halves so stage 2 attends over the full
sequence.

v4 (vs v2 baseline):
  - Softmax weights quantized to fp8 on the ACT-routed score groups
    (e5m2 stage 1 / e4m3 stage 2, with a fixed exp offset c per stage so
    the weight range fits fp8; the offset cancels in the softmax ratio).
    attn@V for those blocks runs as fp8 DoubleRow matmuls (2 key blocks
    per PE pass) -> ~2x AV throughput. V is fp8-e4m3 for DoubleRow
    blocks, bf16 for the DVE-routed (Schraudolph) blocks; numerator and
    denominator share the quantized weights (numpy-validated ~6e-3).
  - Score groups are 2 key blocks; exp routing alternates ACT/DVE per
    group so both engines stream the softmax concurrently and the sA
    PSUM ring (2 bufs) never waits on a backed-up engine queue.
  - DVE runs (almost) only exp + PSUM drains; ELU-relu and the 1/Z
    drain moved to ACT, the ELU-combine and 1/Z broadcast-multiply to
    the Pool engine (SBUF-only operands).
  - The last chunk's MLP is deferred into the NEXT stage's chunk-0 loop
    (pre_mlp hook) and stage-2 chunk 0 processes key blocks in a
    permuted slice order so the late AllGather slices (3, 7) are needed
    last: no cross-stage pipeline bubble.
"""

import numpy as np
from contextlib import ExitStack

import concourse.bass as bass
import concourse.tile as tile
from concourse import bacc, mybir
from concourse import bass_utils

F32 = mybir.dt.float32
F32R = mybir.dt.float32r
BF16 = mybir.dt.bfloat16
FP8E4 = mybir.dt.float8e4
FP8E5 = mybir.dt.float8e5
I32 = mybir.dt.int32
I16 = mybir.dt.int16
EXP = mybir.ActivationFunctionType.Exp
RELU = mybir.ActivationFunctionType.Relu
ADD = mybir.AluOpType.add
MIN = mybir.AluOpType.min
MAX = mybir.AluOpType.max
MULT = mybir.AluOpType.mult
DR = mybir.MatmulPerfMode.DoubleRow

# Schraudolph fast-exp: exp(x) ~ bitcast_f32(i32(A*x + B)); ~3% sawtooth
# rel err per weight, which softmax averaging damps on the output.
SCH_A = 12102203.1616 / 65536.0
SCH_B = 1064986823.0 / 65536.0
# Score-exp groups routed to DVE (Schraudolph/bf16): odd groups; even
# groups go to ACT (exp -> fp8 -> DoubleRow AV). The odd/even split is
# invariant under the slice-level block permutation below.
DVE_GIS = tuple(range(1, 16, 2))
# exp offset per stage: weights are exp(score - c). Stage-1 scores reach
# ~14.4 -> c=4 keeps exp <= 3.3e4 (e5m2 max 57344). Stage-2 scores are
# tiny (max ~0.75) -> c=0 and e4m3 weights.
C_OFF = (4.0, 0.0)
WDT = (FP8E5, FP8E4)

N_CORES = 8
B, S, D = 4, 4096, 64
R = S // 2            # own query rows per core
HD = 256
NCK = R // 512        # si-chunks per core (4 x 512)
NJB = S // 128        # key blocks (32 x 128)
NG = NJB // 2         # score groups per chunk (16 x 2 blocks)
VP = 80               # padded fp8 vA row pitch (DoubleRow needs step%16==0)

# f32r weight pack (col offsets in f32 words)
# region A (partitions 0-63, one 576-col block per stage): wq_dup|wk_dup|wv|w1t
WQ, WK, WV, W1T = 0, 128, 256, 320
RA = 576
W2T0 = 2 * RA                    # region B: [128, 128] per stage
WB = W2T0 + 256
# f32 bias pack
B1C0 = 0                         # [128, 2] per stage -> cols 0..3
B2C1 = 4                         # [64, 1] stage-1 b2_eff (per-partition)
B2R2 = 5                         # [128, 64] stage-2 b2_eff (replicated)
NEGC = B2R2 + 64                 # [128, 2] -C_OFF per stage
BF32C = NEGC + 2


def build_nc(n_cores=N_CORES, reps=1, exch_chunks=NCK):
    nc = bacc.Bacc("TRN2", target_bir_lowering=False, debug=False,
                   num_devices=n_cores)

    xT_d = nc.dram_tensor("xT", [64, S], F32R, kind="ExternalInput").ap()
    w_d = nc.dram_tensor("wpack", [128, WB], F32R, kind="ExternalInput").ap()
    b_d = nc.dram_tensor("bias32", [128, BF32C], F32,
                         kind="ExternalInput").ap()
    out_d = nc.dram_tensor("out1", [R, 64], F32, kind="ExternalOutput").ap()

    with tile.TileContext(nc) as tc, ExitStack() as ctx:
        consts = ctx.enter_context(tc.tile_pool(name="consts", bufs=1))
        sb = ctx.enter_context(tc.tile_pool(name="sb", bufs=1))
        ps = ctx.enter_context(tc.tile_pool(name="ps", bufs=2, space="PSUM"))
        dram = ctx.enter_context(tc.tile_pool(name="dram", bufs=1,
                                              space="DRAM"))

        wt = consts.tile([128, WB], F32R)
        nc.sync.dma_start(wt[:, 0:RA], w_d[:, 0:RA])
        nc.gpsimd.dma_start(wt[:, RA:WB], w_d[:, RA:WB])
        bt = consts.tile([128, BF32C], F32)
        nc.gpsimd.dma_start(bt[:], b_d[:])
        dma_engines = [nc.sync, nc.gpsimd, nc.sync]

        for _rep in range(reps):
            _body(nc, sb, ps, dram, wt, bt, dma_engines,
                  xT_d, out_d, _rep, n_cores)

    nc.compile()
    return nc


def _body(nc, sb, ps, dram, wt, bt, dma_engines, xT_d, out_d, rep, n_cores):
    xT = sb.tile([64, S], F32R, tag="xt", bufs=2, name=f"xT_{rep}")
    for n in range(8):
        dma_engines[n % 3].dma_start(xT[:, n * 512:(n + 1) * 512],
                                     xT_d[:, n * 512:(n + 1) * 512])
    outT = sb.tile([64, R], F32R, tag="outT", bufs=2, name=f"outT_{rep}")
    xT2 = sb.tile([64, S], F32R, tag="xt", bufs=2, name=f"xT2_{rep}")

    def alloc_proj(sfx):
        # qT/kT duplicated across partition halves (row-packed score tiles)
        qT = sb.tile([128, R], F32R, tag=f"qT{sfx}", name=f"qT{sfx}_{rep}")
        kT = sb.tile([128, S], F32R, tag=f"kT{sfx}", name=f"kT{sfx}_{rep}")
        vA8 = sb.tile([128, NJB, VP], FP8E4, tag=f"vA8{sfx}",
                      name=f"vA8{sfx}_{rep}")
        vA16 = sb.tile([128, NJB, 65], BF16, tag=f"vA16{sfx}",
                       name=f"vA16{sfx}_{rep}")
        nc.vector.memset(vA8[:, :, 64:65], 1.0)
        nc.vector.memset(vA16[:, :, 64:65], 1.0)
        return qT, kT, vA8, vA16

    qT1, kT1, vA81, vA161 = alloc_proj(0)
    qT2, kT2, vA82, vA162 = alloc_proj(1)

    def is_dve(b):
        # block b's exp route; invariant across chunks (incl. the permuted
        # stage-2 chunk 0): group parity = (b mod 4) // 2.
        return (b % 4) >= 2

    # --- projection emitters ------------------------------------------------
    def emit_k(sfx, kT, src, sl):
        wsl = wt[0:64, sfx * RA:(sfx + 1) * RA]
        pk = ps.tile([128, 512], F32, tag="mlp", bufs=2)
        nc.tensor.matmul(pk[:], wsl[:, WK:WK + 128],
                         src[:, sl * 512:(sl + 1) * 512],
                         start=True, stop=True)
        nc.scalar.copy(kT[:, sl * 512:(sl + 1) * 512], pk[:])

    def emit_q(sfx, qT, src, sl):
        wsl = wt[0:64, sfx * RA:(sfx + 1) * RA]
        pq = ps.tile([128, 512], F32, tag="mlp", bufs=2)
        nc.tensor.matmul(pq[:], wsl[:, WQ:WQ + 128],
                         src[:, sl * 512:(sl + 1) * 512],
                         start=True, stop=True)
        nc.vector.tensor_copy(qT[:, sl * 512:(sl + 1) * 512], pq[:])

    def emit_v(sfx, vA8, vA16, src, sl, jb0):
        wsl = wt[0:64, sfx * RA:(sfx + 1) * RA]
        pv = ps.tile([128, 4, 64], F32, tag="mlp", bufs=2)
        for b in range(4):
            nc.tensor.matmul(pv[:, b, :],
                             src[:, sl * 512 + b * 128:sl * 512 + (b + 1) * 128],
                             wsl[:, WV:WV + 64], start=True, stop=True)
        # halves of the quad route to fp8 (ACT/DoubleRow) or bf16 (DVE)
        nc.vector.tensor_copy(vA8[:, jb0:jb0 + 2, 0:64], pv[:, 0:2, :])
        nc.vector.tensor_copy(vA16[:, jb0 + 2:jb0 + 4, 0:64], pv[:, 2:4, :])

    def proj_unit(sfx, qT, kT, vA8, vA16, src, sl, with_q=True):
        def fn():
            emit_k(sfx, kT, src, sl)
            emit_v(sfx, vA8, vA16, src, sl, 4 * sl)
            if with_q and sl < NCK:
                emit_q(sfx, qT, src, sl)
        return fn

    # --- stage-1 -> stage-2 exchange ---------------------------------------
    bounce_ins = [dram.tile([64, 512], F32R, name=f"bi_{rep}_{n}",
                            tag=f"bi{n}") for n in range(NCK)]
    bounce_outs = [dram.tile([2, 64, 512], F32R, name=f"bo_{rep}_{n}",
                             tag=f"bo{n}") for n in range(NCK)]

    def exchange(n):
        nc.sync.dma_start(bounce_ins[n][:], outT[:, n * 512:(n + 1) * 512])
        if n_cores > 1:
            nc.gpsimd.collective_compute(
                "AllGather", mybir.AluOpType.bypass,
                replica_groups=[[0, 1], [2, 3], [4, 5], [6, 7]],
                ins=[bounce_ins[n][:].opt()],
                outs=[bounce_outs[n][:].opt()])
        else:
            for m in range(2):
                nc.sync.dma_start(bounce_outs[n][m], bounce_ins[n][:])
        for m in range(2):
            dma_engines[(m * NCK + n) % 3].dma_start(
                xT2[:, m * R + n * 512:m * R + (n + 1) * 512],
                bounce_outs[n][m])

    # --- one attention+MLP stage -------------------------------------------
    def stage(sfx, qT, kT, vA8, vA16, group_emits, write_out, after_mlp,
              pre_mlp=None, order0=None, defer_last=False):
        wsl = wt[0:64, sfx * RA:(sfx + 1) * RA]
        w2t = wt[:, W2T0 + sfx * 128:W2T0 + (sfx + 1) * 128]
        negc = bt[:, NEGC + sfx:NEGC + sfx + 1]
        schb = SCH_B - SCH_A * C_OFF[sfx]
        wdt = WDT[sfx]
        aTs = [None] * NCK

        def mlp(n):
            # elu(x)+1 = max(x,0) + min(exp(x),1); bias-adds fused on ACT
            aT = aTs[n]
            r = sb.tile([128, 1024], F32, tag="r", bufs=2)
            e = sb.tile([128, 1024], F32, tag="e", bufs=2)
            em = sb.tile([128, 1024], F32, tag="em", bufs=2)
            hT = sb.tile([128, 1024], F32R, tag="hT", bufs=2)
            for j in range(2):
                ph = ps.tile([128, 512], F32, tag="mlp", bufs=2)
                nc.tensor.matmul(ph[:],
                                 wsl[:, W1T + j * 128:W1T + (j + 1) * 128],
                                 aT[:], start=True, stop=True)
                b1j = bt[:, sfx * 2 + j:sfx * 2 + j + 1]
                jsl = slice(j * 512, (j + 1) * 512)
                nc.scalar.activation(r[:, jsl], ph[:], RELU, bias=b1j)
                nc.scalar.activation(e[:, jsl], ph[:], EXP, bias=b1j)
                nc.gpsimd.tensor_scalar_min(em[:, jsl], e[:, jsl], 1.0)
                nc.gpsimd.tensor_add(hT[:, jsl], em[:, jsl], r[:, jsl])
            write_out(n, hT, w2t)
            if after_mlp is not None:
                after_mlp(n)

        for n in range(NCK):
            order = order0 if (n == 0 and order0 is not None) \
                else list(range(NJB))
            av_box = [None]

            def emit_av(ex, jb, gi):
                if av_box[0] is None:
                    av_box[0] = ps.tile([65, 512], F32, tag="av", bufs=1,
                                        name=f"av_{rep}_{sfx}_{n}")
                start = gi == 0
                stop = gi == NG - 1
                if gi in DVE_GIS:
                    for i in range(2):
                        nc.tensor.matmul(av_box[0][:],
                                         vA16[:, jb + i, :],
                                         ex[:, i, :].bitcast(BF16),
                                         start=start and i == 0,
                                         stop=stop and i == 1)
                else:
                    nc.tensor.matmul(av_box[0][:], vA8[:, jb:jb + 2, 0:65],
                                     ex[:, 0:2, :], start=start, stop=stop,
                                     perf_mode=DR)

            pend = None
            for gi in range(NG):
                jb = order[2 * gi]
                assert order[2 * gi + 1] == jb + 1
                for fn in group_emits.get((n, gi), ()):
                    fn()
                st = ps.tile([128, 2, 512], F32, tag="sA", bufs=3)
                for i in range(2):
                    h = (jb + i) % 2
                    nc.tensor.matmul(
                        st[:, i, :],
                        kT[h * 64:(h + 1) * 64,
                           (jb + i) * 128:(jb + i + 1) * 128],
                        qT[h * 64:(h + 1) * 64, n * 512:(n + 1) * 512],
                        start=True, stop=True, tile_position=(h * 64, 0))
                if gi in DVE_GIS:
                    exi = sb.tile([128, 2, 512], I16, tag="expi", bufs=2)
                    nc.vector.tensor_scalar(exi[:], st[:], SCH_A, schb,
                                            op0=MULT, op1=ADD)
                    pend_t = (exi, jb, gi)
                else:
                    ex = sb.tile([128, 2, 512], wdt, tag="exp", bufs=3)
                    nc.scalar.activation(ex[:], st[:], EXP, bias=negc)
                    pend_t = (ex, jb, gi)
                if gi == 3:
                    if n > 0:
                        mlp(n - 1)
                    elif pre_mlp is not None:
                        pre_mlp()
                if pend is not None:
                    emit_av(*pend)
                pend = pend_t
            emit_av(*pend)
            av = av_box[0]

            # normalize: aT = av[0:64] / av[64]
            rs = sb.tile([1, 512], F32, tag="rs", bufs=2)
            nc.vector.tensor_copy(rs[:], av[64:65, :])
            rr = sb.tile([1, 512], F32, tag="rr", bufs=2)
            nc.vector.reciprocal_approx_fast(rr[:], rs[:])
            rb = sb.tile([64, 512], F32, tag="rb", bufs=2)
            nc.gpsimd.partition_broadcast(rb[:], rr[:])
            araw = sb.tile([64, 512], F32, tag="araw", bufs=2)
            nc.scalar.copy(araw[:], av[0:64, :])
            aT = sb.tile([64, 512], F32R, tag="aT", bufs=2)
            nc.gpsimd.tensor_mul(aT[:], araw[:], rb[:])
            aTs[n] = aT
        if defer_last:
            return lambda: mlp(NCK - 1)
        mlp(NCK - 1)
        return None

    # --- stage 1 ------------------------------------------------------------
    def write_out1(n, hT, w2t):
        po = ps.tile([64, 512], F32, tag="mlp", bufs=2)
        nc.tensor.matmul(po[:], w2t[:, 0:64], hT[:, 0:512],
                         start=True, stop=False)
        nc.tensor.matmul(po[:], w2t[:, 64:128], hT[:, 512:1024],
                         start=False, stop=True)
        nc.vector.tensor_scalar_add(outT[:, n * 512:(n + 1) * 512], po[:],
                                    bt[0:64, B2C1:B2C1 + 1])

    def after_mlp1(n):
        exchange(n)
        emit_q(1, qT2, outT, n)

    # bootstrap: first two projection slices before the chunk loop, the rest
    # interleaved into chunk 0's groups.
    proj_unit(0, qT1, kT1, vA81, vA161, xT, 0)()
    proj_unit(0, qT1, kT1, vA81, vA161, xT, 1)()
    emits1 = {(0, g): [proj_unit(0, qT1, kT1, vA81, vA161, xT, g + 2)]
              for g in range(6)}
    mlp1_last = stage(0, qT1, kT1, vA81, vA161, emits1, write_out1,
                      after_mlp1, defer_last=True)

    # --- stage 2 ------------------------------------------------------------
    def write_out2(n, hT, w2t):
        for ss in range(4):
            po2 = ps.tile([128, 64], F32, tag="mlp", bufs=2)
            nc.tensor.matmul(po2[:], hT[:, ss * 128:(ss + 1) * 128],
                             w2t[:, 0:64], start=True, stop=False)
            nc.tensor.matmul(po2[:], hT[:, 512 + ss * 128:512 + (ss + 1) * 128],
                             w2t[:, 64:128], start=False, stop=True)
            fin = sb.tile([128, 64], F32, tag="fin", bufs=2)
            nc.vector.tensor_add(fin[:], po2[:], bt[:, B2R2:B2R2 + 64])
            row0 = n * 512 + ss * 128
            nc.sync.dma_start(out_d[row0:row0 + 128, :], fin[:])

    # stage-1's last-chunk MLP runs inside stage-2's chunk-0 pipeline (at
    # gi==3), which also emits exchange(3). Chunk 0 therefore processes the
    # key blocks slice-permuted so slices 3 and 7 (delivered by that late
    # exchange) are needed last.
    SL_ORDER = [0, 1, 2, 4, 5, 6, 3, 7]
    order0 = [4 * s + i for s in SL_ORDER for i in range(4)]
    proj_unit(1, qT2, kT2, vA82, vA162, xT2, 0, with_q=False)()
    proj_unit(1, qT2, kT2, vA82, vA162, xT2, 1, with_q=False)()
    pslot = {0: 2, 1: 4, 2: 5, 4: 6, 5: 3, 6: 7}
    emits2 = {(0, g): [proj_unit(1, qT2, kT2, vA82, vA162, xT2, sl,
                                 with_q=False)]
              for g, sl in pslot.items()}
    stage(1, qT2, kT2, vA82, vA162, emits2, write_out2, None,
          pre_mlp=mlp1_last, order0=order0)


def prep_inputs(x, q, k, v, q1, k1, v1, W1, b1, W2, b2, W11, b11, W22, b22):
    """Returns per-core in_maps for run_bass_kernel_spmd."""
    f = np.float32

    def cast(a):
        return np.ascontiguousarray(np.asarray(a), dtype=f)

    scale = f(0.125)                      # 1/sqrt(QD), folded into wq
    wpack = np.zeros((128, WB), dtype=f)
    bias32 = np.zeros((128, BF32C), dtype=f)
    for sfx, (qq, kk, vv, W1_, b1_, W2_, b2_) in enumerate(
            [(q, k, v, W1, b1, W2, b2), (q1, k1, v1, W11, b11, W22, b22)]):
        c0 = RA * sfx
        wq = cast(qq) * scale
        wpack[0:64, c0 + WQ:c0 + WQ + 64] = wq
        wpack[0:64, c0 + WQ + 64:c0 + WQ + 128] = wq
        wpack[0:64, c0 + WK:c0 + WK + 64] = cast(kk)
        wpack[0:64, c0 + WK + 64:c0 + WK + 128] = cast(kk)
        wpack[0:64, c0 + WV:c0 + WV + 64] = cast(vv)
        wpack[0:64, c0 + W1T:c0 + W1T + HD] = cast(W1_).T
        w2T = cast(W2_).T                                 # [HD, 64]
        for j in range(2):
            wpack[:, W2T0 + sfx * 128 + j * 64:
                  W2T0 + sfx * 128 + (j + 1) * 64] = w2T[j * 128:(j + 1) * 128]
            bias32[:, sfx * 2 + j] = cast(b1_)[j * 128:(j + 1) * 128]
        b2e = cast(b2_) - cast(W2_).sum(axis=1)           # ELU +1 fold
        if sfx == 0:
            bias32[0:64, B2C1] = b2e
        else:
            bias32[:, B2R2:B2R2 + 64] = np.tile(b2e[None, :], (128, 1))
        bias32[:, NEGC + sfx] = -C_OFF[sfx]

    in_maps = []
    xc = cast(x)
    for c in range(N_CORES):
        b, h = c // 2, c % 2
        xb = xc[b]                      # [S, 64]
        if h == 1:                      # own half first
            xb = np.concatenate([xb[R:], xb[:R]], axis=0)
        in_maps.append({"xT": np.ascontiguousarray(xb.T),
                        "wpack": wpack, "bias32": bias32})
    return in_maps


_NC_CACHE = None


def kernel(**inputs) -> np.ndarray:
    global _NC_CACHE
    if _NC_CACHE is None:
        _NC_CACHE = build_nc()
    nc = _NC_CACHE
    in_maps = prep_inputs(**inputs)
    res = bass_utils.run_bass_kernel_spmd(nc, in_maps,
                                          core_ids=list(range(N_CORES)))
    out = np.empty((B, S, 64), dtype=np.float32)
    for c in range(N_CORES):
        b, h = c // 2, c % 2
        out[b, h * R:(h + 1) * R, :] = res.results[c]["out1"]
    return out
